# revision 59
# baseline (speedup 1.0000x reference)
"""Bahdanau additive-attention kernel for Trainium2, SPMD across 8 NeuronCores.

Reference computation (all fp32):
    q_proj  = query @ W1_w.T + W1_b            # [D]
    v_proj  = values @ W2_w.T + W2_b           # [T, D]
    weights = softmax(tanh(q_proj + v_proj) * v, axis=0)   # over T
    out     = weights * values                 # [T, D]

Sharding: values is split along T across 8 cores (2048 rows each); W2/W1 are
replicated (shipped pre-transposed + pre-blocked in fp8e4m3, scaled by 64 to
stay out of fp8 subnormals); the softmax denominator (per-column sum of exps)
is AllReduced.  Logits are bounded in [-0.1, 0.1] (tanh * v with |v| <= 0.1)
so the softmax needs no max-subtraction pass.

Per-core device program:
  - Main matmul v_proj^T = W2T @ valuesT runs in fp8 DoubleRow perf mode
    (256-deep contraction per pass, 2x PE throughput): stationary w2 blocks
    [128, 2, 128], moving vt8 tiles [128, 2, 512]; psum = 64*v_proj in
    [d=128 part, t=512 free].
  - ScalarE: tanh(psum/64 + qb[d]) then exp(v[d] * x) with accum_out giving
    the per-partition running sum of exps (softmax denominator) for free.
  - e stored fp16 (unscaled, ~1.0).  Pass 2: outT = e * valuesT(fp16, /64)
    on DVE; a regular fp16 matmul against the diagonal diag(64/S[d])
    transposes back to [t, d] AND applies the softmax normalization in one
    PE op, writing f32 PSUM; evacuation is a pure f32 copy alternating
    Scalar/Vector; out-DMA issues alternate sync/gpsimd.
  - The q-projection matvec also runs fp8 DoubleRow, redundantly per core
    (cheap), scheduled after dj=1's matmuls so its W1 DMA hides behind the
    vt8 load.
"""

import numpy as np

import concourse.bacc as bacc
import concourse.bass as bass
import concourse.tile as tile
from concourse import mybir
from concourse import masks
from concourse.bass_utils import run_bass_kernel_spmd

F32 = mybir.dt.float32
BF16 = mybir.dt.bfloat16
FP16 = mybir.dt.float16
FP8 = mybir.dt.float8e4

D = 2048          # feature dim
T = 16384         # total timesteps
N_CORES = 8
TS = T // N_CORES  # timesteps per core = 2048

W_SCALE = 64.0           # host-side fp8 scale on W1/W2
INV_W_SCALE = 1.0 / W_SCALE


def build_kernel(D=D, TS=TS, n_cores=N_CORES, debug=False):
    DT = D // 128     # d-tiles of 128
    KT = D // 128     # k-tiles of 128
    KT2 = KT // 2     # k-tile PAIRS (DoubleRow consumes 256 contraction rows)
    TC = TS // 512    # t-chunks of 512
    IT = TS // 128    # t-tiles of 128
    GJ = min(4, DT)   # dj per pass-2 group (one 512-wide d-chunk)
    NG = DT // GJ     # number of pass-2 groups
    THW = min(512, TS)   # pass-2 t-chunk width
    NTH = TS // THW
    N_CORES_ = n_cores
    DR = mybir.MatmulPerfMode.DoubleRow

    nc = bacc.Bacc(None, target_bir_lowering=False, debug=debug, num_devices=N_CORES_)

    # Per-core inputs (see make_in_maps for host-side layouts)
    valsT = nc.dram_tensor("valsT", [D, TS], FP16, kind="ExternalInput")
    valsT8 = nc.dram_tensor("valsT8", [KT2, 128, 2, TS], FP8, kind="ExternalInput")
    w2t8 = nc.dram_tensor("w2t8", [DT, 128, KT, 128], FP8, kind="ExternalInput")
    w1t8 = nc.dram_tensor("w1t8", [KT2, 128, 2, D], FP8, kind="ExternalInput")
    qfull = nc.dram_tensor("qfull", [D], F32, kind="ExternalInput")
    w1b = nc.dram_tensor("w1b", [D], F32, kind="ExternalInput")
    w2b = nc.dram_tensor("w2b", [D], F32, kind="ExternalInput")
    vvec = nc.dram_tensor("vvec", [D], F32, kind="ExternalInput")
    out = nc.dram_tensor("out", [TS, D], F32, kind="ExternalOutput")

    with tile.TileContext(nc) as tc:
        with (
            tc.tile_pool(name="const", bufs=1) as const_pool,
            tc.tile_pool(name="vt", bufs=1) as vt_pool,
            tc.tile_pool(name="e", bufs=1) as e_pool,
            tc.tile_pool(name="w2tb", bufs=2) as w2tb_pool,
            tc.tile_pool(name="st", bufs=2) as st_pool,
            tc.tile_pool(name="psum", bufs=6, space="PSUM") as psum_pool,
            tc.tile_pool(name="psum2", bufs=2, space="PSUM") as psum2_pool,
            tc.tile_pool(name="dram", bufs=1, space="DRAM") as dram_pool,
        ):
            # ---------------- constants / small vectors ----------------
            qbv = const_pool.tile([128, DT], F32)    # qb[d] laid out [p, dj]
            vv = const_pool.tile([128, DT], F32)     # v[d]
            rv2 = const_pool.tile([128, DT], F32)    # 2^14 / S[d]
            Sloc = const_pool.tile([128, DT], F32)   # local sum-exp
            b1v = const_pool.tile([128, DT], F32)
            b2v = const_pool.tile([128, DT], F32)
            ident16 = const_pool.tile([128, 128], FP16)
            ones1 = const_pool.tile([1, 128], F32)

            masks.make_identity(nc, ident16[:, :])
            nc.vector.memset(ones1[:, :], 1.0)

            DH = (3 * DT // 4) if DT >= 4 else DT

            # ---------------- warmup collective (absorbs ncfw first-use) ----
            wu_in = dram_pool.tile([1, 32], F32, name="wu_in")
            wu_out = dram_pool.tile([1, 32], F32, name="wu_out")
            wuz = const_pool.tile([1, 32], F32)
            nc.vector.memset(wuz[:, :], 0.0)
            nc.gpsimd.dma_start(wu_in[:, :], wuz[:, :])
            nc.gpsimd.collective_compute(
                "AllReduce", mybir.AluOpType.add,
                replica_groups=[list(range(N_CORES_))],
                ins=[wu_in.opt()], outs=[wu_out.opt()],
            )

            # first two W2T blocks land before the vt8 bulk so dj0 matmuls
            # can start immediately
            w2tb_pre = []
            for i in range(min(2, DT)):
                wpre = w2tb_pool.tile([128, KT, 128], FP8, tag="w2tb",
                                      name=f"w2tbp{i}")
                nc.sync.dma_start(wpre[:, :, :], w2t8[i, :, :, :])
                w2tb_pre.append(wpre)

            # ---------------- vt8 resident load (fp8, matmul moving op) ----
            # vt8[kt2][p, ks, t] = values_s[t, 256*kt2 + 128*ks + p]
            # Loaded in (kt2, tc) chunk order so dj0's matmuls stream at DMA
            # pace.  Manually scoped: closed after pass-1 so its 32KB/part
            # overlays the tail-only pools (osb/outT/mdiag).
            vt8_ctx = tc.tile_pool(name="vt8", bufs=1)
            vt8_pool = vt8_ctx.__enter__()
            vt8_tiles = []
            for kt2 in range(KT2):
                vt8t = vt8_pool.tile([128, 2, TS], FP8, name=f"vt8_{kt2}")
                vt8_tiles.append(vt8t)
            for kt2 in range(KT2):
                for tcq in range(2):
                    hw = TS // 2
                    nc.sync.dma_start(
                        vt8_tiles[kt2][:, :, tcq * hw:(tcq + 1) * hw],
                        valsT8[kt2, :, :, tcq * hw:(tcq + 1) * hw])

            # ---------------- pass 1: matmul + tanh + exp ---------------
            e_tiles = []
            for dj in range(DT):
                e_tiles.append(e_pool.tile([128, TS], FP16, name=f"e{dj}"))

            # vt16 (pass-2 fp16 values) trickle-loaded one tile per dj
            # iteration on gpsimd — needed only from the first pass-2 group.
            vt_tiles = []
            for kt in range(KT):
                vt = vt_pool.tile([128, TS], FP16, name=f"vt{kt}")
                vt_tiles.append(vt)

            qcol = const_pool.tile([128, KT], F32)   # q in [p, kt] layout
            # q in fp8, one value per 16B block: dual-fp8 LDWEIGHTS requires
            # the pair stride to be even and 16B-aligned, so the (ks=0, ks=1)
            # elements of a DoubleRow pair sit 16 bytes apart.
            qcol16 = const_pool.tile([128, KT * 16], FP8)
            nc.scalar.dma_start(qcol[:, :], qfull[:].rearrange("(kt p) -> p kt", p=128))
            nc.vector.tensor_copy(
                qcol16[:, :].rearrange("p (a b) -> p a b", b=16)[:, :, 0:1],
                qcol[:, :].rearrange("p (a b) -> p a b", b=1))
            QW = min(512, D)
            QDC = D // QW

            def emit_w1_loads(w1_pool):
                # W1 streams on scalar (sync carries vt8; gpsimd is poisoned
                # by the warmup collective) in HALF tiles
                # (0.25MB, 0.7us) with bufs=6, h-major so the matvec's pass A
                # (dc 0,1 = h0 halves) consumes loads in arrival order.
                tiles = {}
                for h in range(2):
                    for kt2 in range(KT2):
                        w1tile = w1_pool.tile([128, 2, D // 2], FP8, tag="w1t")
                        nc.scalar.dma_start(
                            w1tile[:, :, :],
                            w1t8[kt2, :, :, h * (D // 2):(h + 1) * (D // 2)])
                        tiles[(kt2, h)] = w1tile
                return tiles

            def emit_matvec(w1_tiles, qrow_pool):
                # q_proj row = sum_kt2 q_pair.T @ W1T[pair] in fp8 DoubleRow
                # (softmax is nearly invariant to the per-column q_proj
                # quantization error), then transpose the row into the
                # per-partition [p, dj] layout.
                # Two dc-passes so the pq accumulators fit in the dedicated
                # 2-bank psum2 pool — sharing the mains' psum pool here
                # couples the matvec to dj0's activations and wedges the
                # whole early phase.
                qrow = qrow_pool.tile([1, D], F32, name="qrow")  # q_proj row
                NDCQ = D // 2 // QW
                for h in range(2):
                    pq_tiles = [psum2_pool.tile([1, QW], F32, name=f"pq{h}{i}",
                                                tag="pT")
                                for i in range(NDCQ)]
                    for kt2 in range(KT2):
                        qpair = qcol16[:, :].rearrange(
                            "p (a b) -> p a b", b=16)[:, 2 * kt2:2 * kt2 + 2, 0:1]
                        half = w1_tiles[(kt2, h)]
                        for dcq in range(NDCQ):
                            nc.tensor.matmul(
                                pq_tiles[dcq][:, :],
                                qpair,
                                half[:, :, dcq * QW:(dcq + 1) * QW],
                                start=(kt2 == 0), stop=(kt2 == KT2 - 1),
                                perf_mode=DR)
                    for dcq in range(NDCQ):
                        dc = h * NDCQ + dcq
                        nc.scalar.activation(
                            qrow[:, dc * QW:(dc + 1) * QW], pq_tiles[dcq][:, :],
                            mybir.ActivationFunctionType.Copy,
                            bias=0.0, scale=INV_W_SCALE)
                pqt = psum2_pool.tile([128, DT], F32, name="pqt", tag="pT")
                for dj in range(DT):
                    nc.tensor.transpose(
                        pqt[:, dj:dj + 1],
                        qrow[:, dj * 128:(dj + 1) * 128], ones1[:, 0:1])
                nc.scalar.copy(qbv[:, :], pqt[:, :])

                # biases / v in [p, dj] layout: elem (p, j) <- dram[128j + p]
                nc.scalar.dma_start(b1v[:, :], w1b[:].rearrange("(j p) -> p j", p=128))
                nc.scalar.dma_start(b2v[:, :], w2b[:].rearrange("(j p) -> p j", p=128))
                nc.scalar.dma_start(vv[:, :], vvec[:].rearrange("(j p) -> p j", p=128))
                nc.vector.tensor_add(b1v[:, :], b1v[:, :], b2v[:, :])
                nc.vector.tensor_add(qbv[:, :], qbv[:, :], b1v[:, :])

            ndma_state = [0]

            def emit_group(djs):
                # pass-2 pipeline for a list of dj tiles (one contiguous
                # output chunk): outT = e * valuesT/64 on DVE (both fp16,
                # e unscaled), then a regular fp16 matmul against the scaled
                # diagonal M_dj = diag(64/S[d]) transposes AND applies the
                # softmax normalization in one PE op, writing f32 PSUM
                # (recycling the pass-1 psum pool — mains are done by now).
                # Evacuation is a pure f32 copy alternating Scalar/Vector
                # into a per-th staging tile; ONE batched DMA per th writes
                # 512 output rows (4 itl blocks) with a single descriptor.
                nj = len(djs)
                d0 = djs[0]
                nitl = THW // 128
                mds = []
                for dj in djs:
                    md = mdiag_pool.tile([128, 128], FP16, tag="md", name="md")
                    nc.vector.tensor_scalar(
                        out=md[:, :], in0=ident16[:, :],
                        scalar1=rv2[:, dj:dj + 1], scalar2=None,
                        op0=mybir.AluOpType.mult)
                    mds.append(md)
                for th in range(NTH):
                    oT = []
                    for jj in range(nj):
                        dj = djs[jj]
                        ot = outT_pool.tile([128, THW], FP16, tag="oT", name="ot")
                        nc.vector.tensor_mul(
                            ot[:, :],
                            e_tiles[dj][:, th * THW:(th + 1) * THW],
                            vt_tiles[dj][:, th * THW:(th + 1) * THW])
                        oT.append(ot)
                    osb = osb_pool.tile([128, nitl * nj * 128], F32,
                                        name="osb", tag="osb")
                    for itl in range(nitl):
                        pso = psum_pool.tile([128, 512], F32, tag="ps",
                                             name="pso")
                        for jj in range(nj):
                            nc.tensor.matmul(
                                pso[:, jj * 128:(jj + 1) * 128],
                                oT[jj][:, itl * 128:(itl + 1) * 128],
                                mds[jj][:, :],
                                start=True, stop=True)
                        ndma_state[0] += 1
                        oslice = osb[:, itl * nj * 128:(itl + 1) * nj * 128]
                        if ndma_state[0] % 2:
                            nc.scalar.copy(oslice, pso[:, :nj * 128])
                        else:
                            nc.vector.tensor_copy(oslice, pso[:, :nj * 128])
                    nc.sync.dma_start(
                        out[th * THW:(th + 1) * THW,
                            d0 * 128:(d0 + nj) * 128].rearrange(
                                "(a p) f -> p a f", p=128),
                        osb[:, :].rearrange("p (a f) -> p a f", a=nitl))

            # sum-exp AllReduce split points, sized so each AR's ~35us ncfw
            # latency hides under remaining pass-1 matmuls or earlier groups'
            # pass-2 work.  Pass-2 groups are lists of dj indices per output
            # chunk.
            if DT >= 16:
                # Two AR parts: A (dj 0-11) triggers mid-pass-1 and lands
                # before the tail starts; B (dj 12-15) triggers at the last
                # exp and lands while G0-G2 (~35us of evac-paced pass-2)
                # still run — near-zero exposed latency.  No mid-mains
                # interleaving: the chip is power-throttled, so overlapping
                # pass-2 with pass-1 just lowers the PE clock.
                ar_parts = [(0, 12), (12, 16)]
                groups = [list(range(4 * g, 4 * g + 4)) for g in range(3)] + \
                         [[12, 13], [14, 15]]
                interleave_at = {}
                readback_at = {12: 0}
                mid_groups = [groups[0], groups[1], groups[2]]
                b_groups = [groups[3], groups[4]]
            else:
                ar_parts = [(0, DH)] + ([(DH, DT)] if DH < DT else [])
                groups = [list(range(g * GJ, (g + 1) * GJ)) for g in range(NG)]
                interleave_at = {}
                readback_at = {}
                mid_groups = [g for g in groups if g[-1] < DH]
                b_groups = [g for g in groups if g[-1] >= DH]

            s_bounce = []
            for pi, (lo, hi) in enumerate(ar_parts):
                sin = dram_pool.tile([128, hi - lo], F32, name=f"s_in{pi}")
                sout = dram_pool.tile([128, hi - lo], F32, name=f"s_out{pi}")
                s_bounce.append((sin, sout))

            def ar_trigger(pi):
                lo, hi = ar_parts[pi]
                sin, sout = s_bounce[pi]
                nc.gpsimd.dma_start(sin[:, :], Sloc[:, lo:hi])
                nc.gpsimd.collective_compute(
                    "AllReduce", mybir.AluOpType.add,
                    replica_groups=[list(range(N_CORES_))],
                    ins=[sin.opt()], outs=[sout.opt()],
                )

            def ar_readback_dma(pi):
                # Inline on gpsimd right after the matching collective: it
                # head-of-line blocks only LATER collective machinery, which
                # is gated on later data anyway.
                lo, hi = ar_parts[pi]
                sin, sout = s_bounce[pi]
                nc.gpsimd.dma_start(rv2[:, lo:hi], sout[:, :])

            def ar_readback_arith(pi):
                # rv2 = 64 / S  (64/S ~ 0.004 keeps the pass-2 diagonal well
                # inside fp16 normals; vt16 is pre-scaled by 1/64 on host).
                # Emitted post-loop so it never head-of-line blocks the
                # vector queue's pass-1 reduces.
                lo, hi = ar_parts[pi]
                nc.vector.tensor_scalar_mul(rv2[:, lo:hi], rv2[:, lo:hi],
                                            0.015625)
                nc.vector.reciprocal(rv2[:, lo:hi], rv2[:, lo:hi])

            def ar_readback(pi):
                ar_readback_dma(pi)
                ar_readback_arith(pi)

            def emit_act(dj, srcs):
                # tanh per 512-wide psum bank, then ONE 2048-wide exp whose
                # accum_out IS the local softmax denominator (written straight
                # into Sloc — no separate reduce).
                st = st_pool.tile([128, TS], FP16, name="st", tag="st")
                for tc_i in range(TC):
                    nc.scalar.activation(
                        st[:, tc_i * 512:(tc_i + 1) * 512], srcs[tc_i][:, :],
                        mybir.ActivationFunctionType.Tanh,
                        bias=qbv[:, dj:dj + 1], scale=INV_W_SCALE,
                    )
                nc.scalar.activation(
                    e_tiles[dj][:, :], st[:, :],
                    mybir.ActivationFunctionType.Exp,
                    bias=0.0, scale=vv[:, dj:dj + 1],
                    accum_out=Sloc[:, dj:dj + 1],
                )
                inloop_parts = ar_parts[:-1] if len(ar_parts) > 1 else ar_parts
                for pi, (lo, hi) in enumerate(inloop_parts):
                    if dj == hi - 1:
                        ar_trigger(pi)
                        if not readback_at:
                            ar_readback(pi)
                if dj in readback_at:
                    ar_readback_dma(readback_at[dj])

            # The q-projection matvec runs FIRST: its W1 tiles stream on
            # gpsimd while sync streams vt8, so the matvec's ~11us on the
            # tensor queue overlaps the vt8 load that dj0 would have waited
            # for anyway.  qbv is ready before dj0's activations, so no
            # PSUM staging is needed.
            w1_ctx = tc.tile_pool(name="w1pool", bufs=6)
            w1_pool = w1_ctx.__enter__()
            qrow_ctx = tc.tile_pool(name="qrowp", bufs=1)
            qrow_pool = qrow_ctx.__enter__()
            w1_tiles = emit_w1_loads(w1_pool)
            emit_matvec(w1_tiles, qrow_pool)
            qrow_ctx.__exit__(None, None, None)
            w1_ctx.__exit__(None, None, None)
            for dj in range(DT):
                if dj < len(w2tb_pre):
                    w2tb = w2tb_pre[dj]
                else:
                    w2tb = w2tb_pool.tile([128, KT, 128], FP8, tag="w2tb",
                                          name="w2tb")
                    nc.sync.dma_start(w2tb[:, :, :], w2t8[dj, :, :, :])
                # vt16 trickle (pass-2 fp16 values): two tiles per dj from
                # dj=2 on, so it never front-runs the vt8/W1 loads.
                if 2 <= dj < 2 + KT // 2:
                    for h in range(2):
                        kt = 2 * (dj - 2) + h
                        nc.sync.dma_start(
                            vt_tiles[kt][:, :],
                            valsT[kt * 128:(kt + 1) * 128, :])
                ps_tiles = [psum_pool.tile([128, 512], F32, tag="ps", name=f"ps{i}")
                            for i in range(TC)]
                # kt2 OUTER: stationary pair reused TC times; dj==0 streams
                # at vt8-DMA pace.  DoubleRow: 256-deep contraction per pass.
                for kt2 in range(KT2):
                    for tc_i in range(TC):
                        nc.tensor.matmul(
                            ps_tiles[tc_i][:, :],
                            w2tb[:, 2 * kt2:2 * kt2 + 2, :],
                            vt8_tiles[kt2][:, :, tc_i * 512:(tc_i + 1) * 512],
                            start=(kt2 == 0),
                            stop=(kt2 == KT2 - 1),
                            perf_mode=DR,
                        )
                emit_act(dj, ps_tiles)
                for g in interleave_at.get(dj, []):
                    emit_group(g)

            vt8_ctx.__exit__(None, None, None)
            osb_ctx = tc.tile_pool(name="osb", bufs=2)
            osb_pool = osb_ctx.__enter__()
            outT_ctx = tc.tile_pool(name="outT", bufs=4)
            outT_pool = outT_ctx.__enter__()
            mdiag_ctx = tc.tile_pool(name="mdiag", bufs=6)
            mdiag_pool = mdiag_ctx.__enter__()

            # ---------------- pass-2 tail ---------------
            # Trigger the final AR part right after dj15's local reduce,
            # then run the groups as their parts land: A1 (triggered at dj7)
            # is long done, A2 (dj11) lands while G0/G1 run, A3 lands while
            # G2 runs.
            if len(ar_parts) > 1:
                ar_trigger(len(ar_parts) - 1)

            if readback_at:
                ar_readback_arith(0)
                for g in mid_groups:
                    emit_group(g)
                ar_readback(len(ar_parts) - 1)
                for g in b_groups:
                    emit_group(g)
            else:
                if len(ar_parts) > 1:
                    ar_readback(len(ar_parts) - 1)
                for g in mid_groups + b_groups:
                    emit_group(g)

            mdiag_ctx.__exit__(None, None, None)
            outT_ctx.__exit__(None, None, None)
            osb_ctx.__exit__(None, None, None)

    nc.compile()
    return nc


_NC_CACHE = None


def _get_nc():
    global _NC_CACHE
    if _NC_CACHE is None:
        _NC_CACHE = build_kernel()
    return _NC_CACHE


def make_in_maps(query, values, v, W1_w, W1_b, W2_w, W2_b,
                 D_=None, TS_=None, n_cores=N_CORES):
    import ml_dtypes
    D_ = D_ or D
    TS_ = TS_ or TS
    DT_ = D_ // 128
    KT_ = D_ // 128
    KT2_ = KT_ // 2
    fp8 = ml_dtypes.float8_e4m3
    # W1T DoubleRow pairs: [kt2, p, ks, d] = 64*W1_w[d, 256kt2+128ks+p]
    w1t_blocked = np.ascontiguousarray(
        (W1_w.T * W_SCALE).reshape(KT2_, 2, 128, D_).transpose(0, 2, 1, 3)
        .astype(fp8))
    # w2t blocked: B[dj, p, kt, f] = 64*W2_w[128dj+f, 128kt+p]
    # (pairs of k-blocks are adjacent along the kt dim => DoubleRow-ready)
    w2t_blocked = np.ascontiguousarray(
        (W2_w * W_SCALE).reshape(DT_, 128, KT_, 128).transpose(0, 3, 2, 1)
        .astype(fp8))
    in_maps = []
    for c in range(n_cores):
        vs = np.ascontiguousarray(values[c * TS_:(c + 1) * TS_])
        # vt16 carries values/64 (exact power-of-2 scale); the pass-2
        # diagonal is 64/S so the product is values * w.
        vsT = np.ascontiguousarray((vs.T * (1.0 / 64.0)).astype(np.float16))
        vsT8 = np.ascontiguousarray(
            vs.T.astype(fp8).reshape(KT2_, 2, 128, TS_).transpose(0, 2, 1, 3))
        in_maps.append({
            "valsT": vsT,
            "valsT8": vsT8,
            "w2t8": w2t_blocked,
            "w1t8": w1t_blocked,
            "qfull": query,
            "w1b": W1_b,
            "w2b": W2_b,
            "vvec": v,
        })
    return in_maps


def kernel(query, values, v, W1_w, W1_b, W2_w, W2_b, _trace=False, _trace_kwargs=None):
    query = np.asarray(query, np.float32)
    values = np.asarray(values, np.float32)
    v = np.asarray(v, np.float32)
    W1_w = np.asarray(W1_w, np.float32)
    W1_b = np.asarray(W1_b, np.float32)
    W2_w = np.asarray(W2_w, np.float32)
    W2_b = np.asarray(W2_b, np.float32)

    nc = _get_nc()
    in_maps = make_in_maps(query, values, v, W1_w, W1_b, W2_w, W2_b)
    res = run_bass_kernel_spmd(
        nc, in_maps, core_ids=list(range(N_CORES)),
        trace=_trace, **(_trace_kwargs or {}),
    )
    shards = [np.asarray(om["out"], np.float32) for om in res.results]
    out = np.concatenate(shards, axis=0)
    if _trace:
        return out, res
    return out


# revision 62
# speedup vs baseline: 1.0243x; 1.0243x over previous
"""Bahdanau additive-attention kernel for Trainium2, SPMD across 8 NeuronCores.

Reference computation (all fp32):
    q_proj  = query @ W1_w.T + W1_b            # [D]
    v_proj  = values @ W2_w.T + W2_b           # [T, D]
    weights = softmax(tanh(q_proj + v_proj) * v, axis=0)   # over T
    out     = weights * values                 # [T, D]

Sharding: values is split along T across 8 cores (2048 rows each); W2/W1 are
replicated (shipped pre-transposed + pre-blocked in fp8e4m3, scaled by 64 to
stay out of fp8 subnormals); the softmax denominator (per-column sum of exps)
is AllReduced.  Logits are bounded in [-0.1, 0.1] (tanh * v with |v| <= 0.1)
so the softmax needs no max-subtraction pass.

Per-core device program:
  - Main matmul v_proj^T = W2T @ valuesT runs in fp8 DoubleRow perf mode
    (256-deep contraction per pass, 2x PE throughput): stationary w2 blocks
    [128, 2, 128], moving vt8 tiles [128, 2, 512]; psum = 64*v_proj in
    [d=128 part, t=512 free].
  - ScalarE: tanh(psum/64 + qb[d]) then exp(v[d] * x) with accum_out giving
    the per-partition running sum of exps (softmax denominator) for free.
  - e stored fp16 (unscaled, ~1.0).  Pass 2: outT = e * valuesT(fp16, /64)
    on DVE; a regular fp16 matmul against the diagonal diag(64/S[d])
    transposes back to [t, d] AND applies the softmax normalization in one
    PE op, writing f32 PSUM; evacuation is a pure f32 copy alternating
    Scalar/Vector; out-DMA issues alternate sync/gpsimd.
  - The q-projection matvec also runs fp8 DoubleRow, redundantly per core
    (cheap), scheduled after dj=1's matmuls so its W1 DMA hides behind the
    vt8 load.
"""

import numpy as np

import concourse.bacc as bacc
import concourse.bass as bass
import concourse.tile as tile
from concourse import mybir
from concourse import masks
from concourse.bass_utils import run_bass_kernel_spmd

F32 = mybir.dt.float32
BF16 = mybir.dt.bfloat16
FP16 = mybir.dt.float16
FP8 = mybir.dt.float8e4

D = 2048          # feature dim
T = 16384         # total timesteps
N_CORES = 8
TS = T // N_CORES  # timesteps per core = 2048

W_SCALE = 64.0           # host-side fp8 scale on W1/W2
INV_W_SCALE = 1.0 / W_SCALE


def build_kernel(D=D, TS=TS, n_cores=N_CORES, debug=False):
    DT = D // 128     # d-tiles of 128
    KT = D // 128     # k-tiles of 128
    KT2 = KT // 2     # k-tile PAIRS (DoubleRow consumes 256 contraction rows)
    TC = TS // 512    # t-chunks of 512
    IT = TS // 128    # t-tiles of 128
    GJ = min(4, DT)   # dj per pass-2 group (one 512-wide d-chunk)
    NG = DT // GJ     # number of pass-2 groups
    THW = min(512, TS)   # pass-2 t-chunk width
    NTH = TS // THW
    N_CORES_ = n_cores
    DR = mybir.MatmulPerfMode.DoubleRow

    nc = bacc.Bacc(None, target_bir_lowering=False, debug=debug, num_devices=N_CORES_)

    # Per-core inputs (see make_in_maps for host-side layouts)
    valsT = nc.dram_tensor("valsT", [D, TS], FP16, kind="ExternalInput")
    valsT8 = nc.dram_tensor("valsT8", [KT2, 128, 2, TS], FP8, kind="ExternalInput")
    w2t8 = nc.dram_tensor("w2t8", [DT, 128, KT, 128], FP8, kind="ExternalInput")
    w1t8 = nc.dram_tensor("w1t8", [KT2, 128, 2, D], FP8, kind="ExternalInput")
    qfull = nc.dram_tensor("qfull", [D], F32, kind="ExternalInput")
    w1b = nc.dram_tensor("w1b", [D], F32, kind="ExternalInput")
    w2b = nc.dram_tensor("w2b", [D], F32, kind="ExternalInput")
    vvec = nc.dram_tensor("vvec", [D], F32, kind="ExternalInput")
    out = nc.dram_tensor("out", [TS, D], F32, kind="ExternalOutput")

    with tile.TileContext(nc) as tc:
        with (
            tc.tile_pool(name="const", bufs=1) as const_pool,
            tc.tile_pool(name="vt", bufs=1) as vt_pool,
            tc.tile_pool(name="e", bufs=1) as e_pool,
            tc.tile_pool(name="w2tb", bufs=2) as w2tb_pool,
            tc.tile_pool(name="st", bufs=2) as st_pool,
            tc.tile_pool(name="psum", bufs=6, space="PSUM") as psum_pool,
            tc.tile_pool(name="psum2", bufs=2, space="PSUM") as psum2_pool,
            tc.tile_pool(name="dram", bufs=1, space="DRAM") as dram_pool,
        ):
            # ---------------- constants / small vectors ----------------
            qbv = const_pool.tile([128, DT], F32)    # qb[d] laid out [p, dj]
            vv = const_pool.tile([128, DT], F32)     # v[d]
            rv2 = const_pool.tile([128, DT], F32)    # 2^14 / S[d]
            Sloc = const_pool.tile([128, DT], F32)   # local sum-exp
            b1v = const_pool.tile([128, DT], F32)
            b2v = const_pool.tile([128, DT], F32)
            ident16 = const_pool.tile([128, 128], FP16)
            ones1 = const_pool.tile([1, 128], F32)

            masks.make_identity(nc, ident16[:, :])
            nc.vector.memset(ones1[:, :], 1.0)

            DH = (3 * DT // 4) if DT >= 4 else DT

            # ---------------- warmup collective (absorbs ncfw first-use) ----
            wu_in = dram_pool.tile([1, 32], F32, name="wu_in")
            wu_out = dram_pool.tile([1, 32], F32, name="wu_out")
            wuz = const_pool.tile([1, 32], F32)
            nc.vector.memset(wuz[:, :], 0.0)
            nc.gpsimd.dma_start(wu_in[:, :], wuz[:, :])
            nc.gpsimd.collective_compute(
                "AllReduce", mybir.AluOpType.add,
                replica_groups=[list(range(N_CORES_))],
                ins=[wu_in.opt()], outs=[wu_out.opt()],
            )

            # first two W2T blocks land before the vt8 bulk so dj0 matmuls
            # can start immediately
            w2tb_pre = []
            for i in range(min(2, DT)):
                wpre = w2tb_pool.tile([128, KT, 128], FP8, tag="w2tb",
                                      name=f"w2tbp{i}")
                nc.sync.dma_start(wpre[:, :, :], w2t8[i, :, :, :])
                w2tb_pre.append(wpre)

            # ---------------- vt8 resident load (fp8, matmul moving op) ----
            # vt8[kt2][p, ks, t] = values_s[t, 256*kt2 + 128*ks + p]
            # Loaded in (kt2, tc) chunk order so dj0's matmuls stream at DMA
            # pace.  Manually scoped: closed after pass-1 so its 32KB/part
            # overlays the tail-only pools (osb/outT/mdiag).
            vt8_ctx = tc.tile_pool(name="vt8", bufs=1)
            vt8_pool = vt8_ctx.__enter__()
            vt8_tiles = []
            for kt2 in range(KT2):
                vt8t = vt8_pool.tile([128, 2, TS], FP8, name=f"vt8_{kt2}")
                vt8_tiles.append(vt8t)
            for kt2 in range(KT2):
                for tcq in range(2):
                    hw = TS // 2
                    nc.sync.dma_start(
                        vt8_tiles[kt2][:, :, tcq * hw:(tcq + 1) * hw],
                        valsT8[kt2, :, :, tcq * hw:(tcq + 1) * hw])

            # ---------------- pass 1: matmul + tanh + exp ---------------
            e_tiles = []
            for dj in range(DT):
                e_tiles.append(e_pool.tile([128, TS], FP16, name=f"e{dj}"))

            # vt16 (pass-2 fp16 values) trickle-loaded one tile per dj
            # iteration on gpsimd — needed only from the first pass-2 group.
            vt_tiles = []
            for kt in range(KT):
                vt = vt_pool.tile([128, TS], FP16, name=f"vt{kt}")
                vt_tiles.append(vt)

            qcol = const_pool.tile([128, KT], F32)   # q in [p, kt] layout
            # q in fp8, one value per 16B block: dual-fp8 LDWEIGHTS requires
            # the pair stride to be even and 16B-aligned, so the (ks=0, ks=1)
            # elements of a DoubleRow pair sit 16 bytes apart.
            qcol16 = const_pool.tile([128, KT * 16], FP8)
            nc.scalar.dma_start(qcol[:, :], qfull[:].rearrange("(kt p) -> p kt", p=128))
            nc.vector.tensor_copy(
                qcol16[:, :].rearrange("p (a b) -> p a b", b=16)[:, :, 0:1],
                qcol[:, :].rearrange("p (a b) -> p a b", b=1))
            QW = min(512, D)
            QDC = D // QW

            def emit_w1_loads(w1_pool):
                # W1 streams on scalar (sync carries vt8; gpsimd is poisoned
                # by the warmup collective) in HALF tiles
                # (0.25MB, 0.7us) with bufs=6, h-major so the matvec's pass A
                # (dc 0,1 = h0 halves) consumes loads in arrival order.
                tiles = {}
                for h in range(2):
                    for kt2 in range(KT2):
                        w1tile = w1_pool.tile([128, 2, D // 2], FP8, tag="w1t")
                        nc.scalar.dma_start(
                            w1tile[:, :, :],
                            w1t8[kt2, :, :, h * (D // 2):(h + 1) * (D // 2)])
                        tiles[(kt2, h)] = w1tile
                return tiles

            def emit_matvec(w1_tiles, qrow_pool):
                # q_proj row = sum_kt2 q_pair.T @ W1T[pair] in fp8 DoubleRow
                # (softmax is nearly invariant to the per-column q_proj
                # quantization error), then transpose the row into the
                # per-partition [p, dj] layout.
                # Two dc-passes so the pq accumulators fit in the dedicated
                # 2-bank psum2 pool — sharing the mains' psum pool here
                # couples the matvec to dj0's activations and wedges the
                # whole early phase.
                qrow = qrow_pool.tile([1, D], F32, name="qrow")  # q_proj row
                NDCQ = D // 2 // QW
                for h in range(2):
                    pq_tiles = [psum2_pool.tile([1, QW], F32, name=f"pq{h}{i}",
                                                tag="pT")
                                for i in range(NDCQ)]
                    for kt2 in range(KT2):
                        qpair = qcol16[:, :].rearrange(
                            "p (a b) -> p a b", b=16)[:, 2 * kt2:2 * kt2 + 2, 0:1]
                        half = w1_tiles[(kt2, h)]
                        for dcq in range(NDCQ):
                            nc.tensor.matmul(
                                pq_tiles[dcq][:, :],
                                qpair,
                                half[:, :, dcq * QW:(dcq + 1) * QW],
                                start=(kt2 == 0), stop=(kt2 == KT2 - 1),
                                perf_mode=DR)
                    for dcq in range(NDCQ):
                        dc = h * NDCQ + dcq
                        nc.scalar.activation(
                            qrow[:, dc * QW:(dc + 1) * QW], pq_tiles[dcq][:, :],
                            mybir.ActivationFunctionType.Copy,
                            bias=0.0, scale=INV_W_SCALE)
                pqt = psum2_pool.tile([128, DT], F32, name="pqt", tag="pT")
                for dj in range(DT):
                    nc.tensor.transpose(
                        pqt[:, dj:dj + 1],
                        qrow[:, dj * 128:(dj + 1) * 128], ones1[:, 0:1])
                nc.scalar.copy(qbv[:, :], pqt[:, :])

                # biases / v in [p, dj] layout: elem (p, j) <- dram[128j + p]
                nc.scalar.dma_start(b1v[:, :], w1b[:].rearrange("(j p) -> p j", p=128))
                nc.scalar.dma_start(b2v[:, :], w2b[:].rearrange("(j p) -> p j", p=128))
                nc.scalar.dma_start(vv[:, :], vvec[:].rearrange("(j p) -> p j", p=128))
                nc.vector.tensor_add(b1v[:, :], b1v[:, :], b2v[:, :])
                nc.vector.tensor_add(qbv[:, :], qbv[:, :], b1v[:, :])

            ndma_state = [0]

            def emit_group(djs, tail3=False):
                # pass-2 pipeline for a list of dj tiles (one contiguous
                # output chunk): outT = e * valuesT/64 on DVE (both fp16,
                # e unscaled), then a regular fp16 matmul against the scaled
                # diagonal M_dj = diag(64/S[d]) transposes AND applies the
                # softmax normalization in one PE op, writing f32 PSUM
                # (recycling the pass-1 psum pool — mains are done by now).
                # Evacuation is a pure f32 copy alternating Scalar/Vector
                # into a per-th staging tile; ONE batched DMA per th writes
                # 512 output rows (4 itl blocks) with a single descriptor.
                nj = len(djs)
                d0 = djs[0]
                nitl = THW // 128
                mds = []
                for dj in djs:
                    md = mdiag_pool.tile([128, 128], FP16, tag="md", name="md")
                    nc.vector.tensor_scalar(
                        out=md[:, :], in0=ident16[:, :],
                        scalar1=rv2[:, dj:dj + 1], scalar2=None,
                        op0=mybir.AluOpType.mult)
                    mds.append(md)
                for th in range(NTH):
                    oT = []
                    for jj in range(nj):
                        dj = djs[jj]
                        ot = outT_pool.tile([128, THW], FP16, tag="oT", name="ot")
                        nc.vector.tensor_mul(
                            ot[:, :],
                            e_tiles[dj][:, th * THW:(th + 1) * THW],
                            vt_tiles[dj][:, th * THW:(th + 1) * THW])
                        oT.append(ot)
                    osb = osb_pool.tile([128, nitl * nj * 128], F32,
                                        name="osb", tag="osb")
                    for itl in range(nitl):
                        pso = psum_pool.tile([128, 512], F32, tag="ps",
                                             name="pso")
                        for jj in range(nj):
                            nc.tensor.matmul(
                                pso[:, jj * 128:(jj + 1) * 128],
                                oT[jj][:, itl * 128:(itl + 1) * 128],
                                mds[jj][:, :],
                                start=True, stop=True)
                        ndma_state[0] += 1
                        oslice = osb[:, itl * nj * 128:(itl + 1) * nj * 128]
                        if ndma_state[0] % 2:
                            nc.scalar.copy(oslice, pso[:, :nj * 128])
                        else:
                            nc.vector.tensor_copy(oslice, pso[:, :nj * 128])
                    nc.sync.dma_start(
                        out[th * THW:(th + 1) * THW,
                            d0 * 128:(d0 + nj) * 128].rearrange(
                                "(a p) f -> p a f", p=128),
                        osb[:, :].rearrange("p (a f) -> p a f", a=nitl))

            # sum-exp AllReduce split points, sized so each AR's ~35us ncfw
            # latency hides under remaining pass-1 matmuls or earlier groups'
            # pass-2 work.  Pass-2 groups are lists of dj indices per output
            # chunk.
            if DT >= 16:
                # Two AR parts: A (dj 0-11) triggers mid-pass-1 and lands
                # before the tail starts; B (dj 12-15) triggers at the last
                # exp and lands while G0-G2 (~35us of evac-paced pass-2)
                # still run — near-zero exposed latency.  No mid-mains
                # interleaving: the chip is power-throttled, so overlapping
                # pass-2 with pass-1 just lowers the PE clock.
                ar_parts = [(0, 12), (12, 16)]
                groups = [list(range(4 * g, 4 * g + 4)) for g in range(3)] + \
                         [[12, 13], [14, 15]]
                interleave_at = {}
                readback_at = {12: 0}
                mid_groups = [groups[0], groups[1], groups[2]]
                b_groups = [groups[3], groups[4]]
            else:
                ar_parts = [(0, DH)] + ([(DH, DT)] if DH < DT else [])
                groups = [list(range(g * GJ, (g + 1) * GJ)) for g in range(NG)]
                interleave_at = {}
                readback_at = {}
                mid_groups = [g for g in groups if g[-1] < DH]
                b_groups = [g for g in groups if g[-1] >= DH]

            s_bounce = []
            for pi, (lo, hi) in enumerate(ar_parts):
                sin = dram_pool.tile([128, hi - lo], F32, name=f"s_in{pi}")
                sout = dram_pool.tile([128, hi - lo], F32, name=f"s_out{pi}")
                s_bounce.append((sin, sout))

            def ar_trigger(pi):
                lo, hi = ar_parts[pi]
                sin, sout = s_bounce[pi]
                nc.gpsimd.dma_start(sin[:, :], Sloc[:, lo:hi])
                nc.gpsimd.collective_compute(
                    "AllReduce", mybir.AluOpType.add,
                    replica_groups=[list(range(N_CORES_))],
                    ins=[sin.opt()], outs=[sout.opt()],
                )

            def ar_readback_dma(pi):
                # Inline on gpsimd right after the matching collective: it
                # head-of-line blocks only LATER collective machinery, which
                # is gated on later data anyway.
                lo, hi = ar_parts[pi]
                sin, sout = s_bounce[pi]
                nc.gpsimd.dma_start(rv2[:, lo:hi], sout[:, :])

            def ar_readback_arith(pi):
                # rv2 = 64 / S  (64/S ~ 0.004 keeps the pass-2 diagonal well
                # inside fp16 normals; vt16 is pre-scaled by 1/64 on host).
                # Emitted post-loop so it never head-of-line blocks the
                # vector queue's pass-1 reduces.
                lo, hi = ar_parts[pi]
                nc.vector.tensor_scalar_mul(rv2[:, lo:hi], rv2[:, lo:hi],
                                            0.015625)
                nc.vector.reciprocal(rv2[:, lo:hi], rv2[:, lo:hi])

            def ar_readback(pi):
                ar_readback_dma(pi)
                ar_readback_arith(pi)

            def emit_act(dj, srcs):
                # tanh per 512-wide psum bank, then ONE 2048-wide exp whose
                # accum_out IS the local softmax denominator (written straight
                # into Sloc — no separate reduce).
                st = st_pool.tile([128, TS], FP16, name="st", tag="st")
                for tc_i in range(TC):
                    nc.scalar.activation(
                        st[:, tc_i * 512:(tc_i + 1) * 512], srcs[tc_i][:, :],
                        mybir.ActivationFunctionType.Tanh,
                        bias=qbv[:, dj:dj + 1], scale=INV_W_SCALE,
                    )
                nc.scalar.activation(
                    e_tiles[dj][:, :], st[:, :],
                    mybir.ActivationFunctionType.Exp,
                    bias=0.0, scale=vv[:, dj:dj + 1],
                    accum_out=Sloc[:, dj:dj + 1],
                )
                inloop_parts = ar_parts[:-1] if len(ar_parts) > 1 else ar_parts
                for pi, (lo, hi) in enumerate(inloop_parts):
                    if dj == hi - 1:
                        ar_trigger(pi)
                        if not readback_at:
                            ar_readback(pi)
                if dj in readback_at:
                    ar_readback_dma(readback_at[dj])

            # The q-projection matvec runs FIRST: its W1 tiles stream on
            # gpsimd while sync streams vt8, so the matvec's ~11us on the
            # tensor queue overlaps the vt8 load that dj0 would have waited
            # for anyway.  qbv is ready before dj0's activations, so no
            # PSUM staging is needed.
            w1_ctx = tc.tile_pool(name="w1pool", bufs=6)
            w1_pool = w1_ctx.__enter__()
            qrow_ctx = tc.tile_pool(name="qrowp", bufs=1)
            qrow_pool = qrow_ctx.__enter__()
            w1_tiles = emit_w1_loads(w1_pool)
            emit_matvec(w1_tiles, qrow_pool)
            qrow_ctx.__exit__(None, None, None)
            w1_ctx.__exit__(None, None, None)
            for dj in range(DT):
                if dj < len(w2tb_pre):
                    w2tb = w2tb_pre[dj]
                else:
                    w2tb = w2tb_pool.tile([128, KT, 128], FP8, tag="w2tb",
                                          name="w2tb")
                    nc.sync.dma_start(w2tb[:, :, :], w2t8[dj, :, :, :])
                # vt16 trickle (pass-2 fp16 values): two tiles per dj from
                # dj=2 on, so it never front-runs the vt8/W1 loads.
                if 2 <= dj < 2 + KT // 2:
                    for h in range(2):
                        kt = 2 * (dj - 2) + h
                        nc.sync.dma_start(
                            vt_tiles[kt][:, :],
                            valsT[kt * 128:(kt + 1) * 128, :])
                ps_tiles = [psum_pool.tile([128, 512], F32, tag="ps", name=f"ps{i}")
                            for i in range(TC)]
                # kt2 OUTER: stationary pair reused TC times; dj==0 streams
                # at vt8-DMA pace.  DoubleRow: 256-deep contraction per pass.
                # The last two djs run tc-OUTER instead, so their psum banks
                # complete (and free) incrementally — the tail's first
                # transpose blocks then don't wait on dj15's last matmul.
                if dj >= DT - 2:
                    for tc_i in range(TC):
                        for kt2 in range(KT2):
                            nc.tensor.matmul(
                                ps_tiles[tc_i][:, :],
                                w2tb[:, 2 * kt2:2 * kt2 + 2, :],
                                vt8_tiles[kt2][:, :, tc_i * 512:(tc_i + 1) * 512],
                                start=(kt2 == 0),
                                stop=(kt2 == KT2 - 1),
                                perf_mode=DR,
                            )
                else:
                    for kt2 in range(KT2):
                        for tc_i in range(TC):
                            nc.tensor.matmul(
                                ps_tiles[tc_i][:, :],
                                w2tb[:, 2 * kt2:2 * kt2 + 2, :],
                                vt8_tiles[kt2][:, :, tc_i * 512:(tc_i + 1) * 512],
                                start=(kt2 == 0),
                                stop=(kt2 == KT2 - 1),
                                perf_mode=DR,
                            )
                emit_act(dj, ps_tiles)
                for g in interleave_at.get(dj, []):
                    emit_group(g)

            vt8_ctx.__exit__(None, None, None)
            osb_ctx = tc.tile_pool(name="osb", bufs=2)
            osb_pool = osb_ctx.__enter__()
            outT_ctx = tc.tile_pool(name="outT", bufs=4)
            outT_pool = outT_ctx.__enter__()
            mdiag_ctx = tc.tile_pool(name="mdiag", bufs=6)
            mdiag_pool = mdiag_ctx.__enter__()

            # ---------------- pass-2 tail ---------------
            # Trigger the final AR part right after dj15's local reduce,
            # then run the groups as their parts land: A1 (triggered at dj7)
            # is long done, A2 (dj11) lands while G0/G1 run, A3 lands while
            # G2 runs.
            if len(ar_parts) > 1:
                ar_trigger(len(ar_parts) - 1)

            if readback_at:
                ar_readback_arith(0)
                for g in mid_groups:
                    emit_group(g)
                ar_readback(len(ar_parts) - 1)
                for g in b_groups:
                    emit_group(g, tail3=True)
            else:
                if len(ar_parts) > 1:
                    ar_readback(len(ar_parts) - 1)
                for g in mid_groups + b_groups:
                    emit_group(g)

            mdiag_ctx.__exit__(None, None, None)
            outT_ctx.__exit__(None, None, None)
            osb_ctx.__exit__(None, None, None)

    nc.compile()
    return nc


_NC_CACHE = None


def _get_nc():
    global _NC_CACHE
    if _NC_CACHE is None:
        _NC_CACHE = build_kernel()
    return _NC_CACHE


def make_in_maps(query, values, v, W1_w, W1_b, W2_w, W2_b,
                 D_=None, TS_=None, n_cores=N_CORES):
    import ml_dtypes
    D_ = D_ or D
    TS_ = TS_ or TS
    DT_ = D_ // 128
    KT_ = D_ // 128
    KT2_ = KT_ // 2
    fp8 = ml_dtypes.float8_e4m3
    # W1T DoubleRow pairs: [kt2, p, ks, d] = 64*W1_w[d, 256kt2+128ks+p]
    w1t_blocked = np.ascontiguousarray(
        (W1_w.T * W_SCALE).reshape(KT2_, 2, 128, D_).transpose(0, 2, 1, 3)
        .astype(fp8))
    # w2t blocked: B[dj, p, kt, f] = 64*W2_w[128dj+f, 128kt+p]
    # (pairs of k-blocks are adjacent along the kt dim => DoubleRow-ready)
    w2t_blocked = np.ascontiguousarray(
        (W2_w * W_SCALE).reshape(DT_, 128, KT_, 128).transpose(0, 3, 2, 1)
        .astype(fp8))
    in_maps = []
    for c in range(n_cores):
        vs = np.ascontiguousarray(values[c * TS_:(c + 1) * TS_])
        # vt16 carries values/64 (exact power-of-2 scale); the pass-2
        # diagonal is 64/S so the product is values * w.
        vsT = np.ascontiguousarray((vs.T * (1.0 / 64.0)).astype(np.float16))
        vsT8 = np.ascontiguousarray(
            vs.T.astype(fp8).reshape(KT2_, 2, 128, TS_).transpose(0, 2, 1, 3))
        in_maps.append({
            "valsT": vsT,
            "valsT8": vsT8,
            "w2t8": w2t_blocked,
            "w1t8": w1t_blocked,
            "qfull": query,
            "w1b": W1_b,
            "w2b": W2_b,
            "vvec": v,
        })
    return in_maps


def kernel(query, values, v, W1_w, W1_b, W2_w, W2_b, _trace=False, _trace_kwargs=None):
    query = np.asarray(query, np.float32)
    values = np.asarray(values, np.float32)
    v = np.asarray(v, np.float32)
    W1_w = np.asarray(W1_w, np.float32)
    W1_b = np.asarray(W1_b, np.float32)
    W2_w = np.asarray(W2_w, np.float32)
    W2_b = np.asarray(W2_b, np.float32)

    nc = _get_nc()
    in_maps = make_in_maps(query, values, v, W1_w, W1_b, W2_w, W2_b)
    res = run_bass_kernel_spmd(
        nc, in_maps, core_ids=list(range(N_CORES)),
        trace=_trace, **(_trace_kwargs or {}),
    )
    shards = [np.asarray(om["out"], np.float32) for om in res.results]
    out = np.concatenate(shards, axis=0)
    if _trace:
        return out, res
    return out


# revision 63
# speedup vs baseline: 1.0748x; 1.0492x over previous
"""Bahdanau additive-attention kernel for Trainium2, SPMD across 8 NeuronCores.

Reference computation (all fp32):
    q_proj  = query @ W1_w.T + W1_b            # [D]
    v_proj  = values @ W2_w.T + W2_b           # [T, D]
    weights = softmax(tanh(q_proj + v_proj) * v, axis=0)   # over T
    out     = weights * values                 # [T, D]

Sharding: values is split along T across 8 cores (2048 rows each); W2/W1 are
replicated (shipped pre-transposed + pre-blocked in fp8e4m3, scaled by 64 to
stay out of fp8 subnormals); the softmax denominator (per-column sum of exps)
is AllReduced.  Logits are bounded in [-0.1, 0.1] (tanh * v with |v| <= 0.1)
so the softmax needs no max-subtraction pass.

Per-core device program:
  - Main matmul v_proj^T = W2T @ valuesT runs in fp8 DoubleRow perf mode
    (256-deep contraction per pass, 2x PE throughput): stationary w2 blocks
    [128, 2, 128], moving vt8 tiles [128, 2, 512]; psum = 64*v_proj in
    [d=128 part, t=512 free].
  - ScalarE: tanh(psum/64 + qb[d]) then exp(v[d] * x) with accum_out giving
    the per-partition running sum of exps (softmax denominator) for free.
  - e stored fp16 (unscaled, ~1.0).  Pass 2: outT = e * valuesT(fp16, /64)
    on DVE; a regular fp16 matmul against the diagonal diag(64/S[d])
    transposes back to [t, d] AND applies the softmax normalization in one
    PE op, writing f32 PSUM; evacuation is a pure f32 copy alternating
    Scalar/Vector; out-DMA issues alternate sync/gpsimd.
  - The q-projection matvec also runs fp8 DoubleRow, redundantly per core
    (cheap), scheduled after dj=1's matmuls so its W1 DMA hides behind the
    vt8 load.
"""

import numpy as np

import concourse.bacc as bacc
import concourse.bass as bass
import concourse.tile as tile
from concourse import mybir
from concourse import masks
from concourse.bass_utils import run_bass_kernel_spmd

F32 = mybir.dt.float32
BF16 = mybir.dt.bfloat16
FP16 = mybir.dt.float16
FP8 = mybir.dt.float8e4

D = 2048          # feature dim
T = 16384         # total timesteps
N_CORES = 8
TS = T // N_CORES  # timesteps per core = 2048

W_SCALE = 64.0           # host-side fp8 scale on W1/W2
INV_W_SCALE = 1.0 / W_SCALE


def build_kernel(D=D, TS=TS, n_cores=N_CORES, debug=False):
    DT = D // 128     # d-tiles of 128
    KT = D // 128     # k-tiles of 128
    KT2 = KT // 2     # k-tile PAIRS (DoubleRow consumes 256 contraction rows)
    TC = TS // 512    # t-chunks of 512
    IT = TS // 128    # t-tiles of 128
    GJ = min(4, DT)   # dj per pass-2 group (one 512-wide d-chunk)
    NG = DT // GJ     # number of pass-2 groups
    THW = min(512, TS)   # pass-2 t-chunk width
    NTH = TS // THW
    N_CORES_ = n_cores
    DR = mybir.MatmulPerfMode.DoubleRow

    nc = bacc.Bacc(None, target_bir_lowering=False, debug=debug, num_devices=N_CORES_)

    # Per-core inputs (see make_in_maps for host-side layouts)
    valsT = nc.dram_tensor("valsT", [D, TS], FP16, kind="ExternalInput")
    valsT8 = nc.dram_tensor("valsT8", [KT2, 128, 2, TS], FP8, kind="ExternalInput")
    w2t8 = nc.dram_tensor("w2t8", [DT, 128, KT, 128], FP8, kind="ExternalInput")
    w1t8 = nc.dram_tensor("w1t8", [KT2, 128, 2, D], FP8, kind="ExternalInput")
    qfull = nc.dram_tensor("qfull", [D], F32, kind="ExternalInput")
    w1b = nc.dram_tensor("w1b", [D], F32, kind="ExternalInput")
    w2b = nc.dram_tensor("w2b", [D], F32, kind="ExternalInput")
    vvec = nc.dram_tensor("vvec", [D], F32, kind="ExternalInput")
    out = nc.dram_tensor("out", [TS, D], F32, kind="ExternalOutput")

    with tile.TileContext(nc) as tc:
        with (
            tc.tile_pool(name="const", bufs=1) as const_pool,
            tc.tile_pool(name="vt", bufs=1) as vt_pool,
            tc.tile_pool(name="e", bufs=1) as e_pool,
            tc.tile_pool(name="w2tb", bufs=2) as w2tb_pool,
            tc.tile_pool(name="st", bufs=2) as st_pool,
            tc.tile_pool(name="psum", bufs=6, space="PSUM") as psum_pool,
            tc.tile_pool(name="psum2", bufs=2, space="PSUM") as psum2_pool,
            tc.tile_pool(name="dram", bufs=1, space="DRAM") as dram_pool,
        ):
            # ---------------- constants / small vectors ----------------
            qbv = const_pool.tile([128, DT], F32)    # qb[d] laid out [p, dj]
            vv = const_pool.tile([128, DT], F32)     # v[d]
            rv2 = const_pool.tile([128, DT], F32)    # 2^14 / S[d]
            Sloc = const_pool.tile([128, DT], F32)   # local sum-exp
            b1v = const_pool.tile([128, DT], F32)
            b2v = const_pool.tile([128, DT], F32)
            ident16 = const_pool.tile([128, 128], FP16)
            ones1 = const_pool.tile([1, 128], F32)

            masks.make_identity(nc, ident16[:, :])
            nc.vector.memset(ones1[:, :], 1.0)

            DH = (3 * DT // 4) if DT >= 4 else DT

            # ---------------- warmup collective (absorbs ncfw first-use) ----
            wu_in = dram_pool.tile([1, 32], F32, name="wu_in")
            wu_out = dram_pool.tile([1, 32], F32, name="wu_out")
            wuz = const_pool.tile([1, 32], F32)
            nc.vector.memset(wuz[:, :], 0.0)
            nc.gpsimd.dma_start(wu_in[:, :], wuz[:, :])
            nc.gpsimd.collective_compute(
                "AllReduce", mybir.AluOpType.add,
                replica_groups=[list(range(N_CORES_))],
                ins=[wu_in.opt()], outs=[wu_out.opt()],
            )

            # first two W2T blocks land before the vt8 bulk so dj0 matmuls
            # can start immediately
            w2tb_pre = []
            for i in range(min(2, DT)):
                wpre = w2tb_pool.tile([128, KT, 128], FP8, tag="w2tb",
                                      name=f"w2tbp{i}")
                nc.sync.dma_start(wpre[:, :, :], w2t8[i, :, :, :])
                w2tb_pre.append(wpre)

            # ---------------- vt8 resident load (fp8, matmul moving op) ----
            # vt8[kt2][p, ks, t] = values_s[t, 256*kt2 + 128*ks + p]
            # Loaded in (kt2, tc) chunk order so dj0's matmuls stream at DMA
            # pace.  Manually scoped: closed after pass-1 so its 32KB/part
            # overlays the tail-only pools (osb/outT/mdiag).
            vt8_ctx = tc.tile_pool(name="vt8", bufs=1)
            vt8_pool = vt8_ctx.__enter__()
            vt8_tiles = []
            for kt2 in range(KT2):
                vt8t = vt8_pool.tile([128, 2, TS], FP8, name=f"vt8_{kt2}")
                vt8_tiles.append(vt8t)
            for kt2 in range(KT2):
                for tcq in range(2):
                    hw = TS // 2
                    nc.sync.dma_start(
                        vt8_tiles[kt2][:, :, tcq * hw:(tcq + 1) * hw],
                        valsT8[kt2, :, :, tcq * hw:(tcq + 1) * hw])

            # ---------------- pass 1: matmul + tanh + exp ---------------
            e_tiles = []
            for dj in range(DT):
                e_tiles.append(e_pool.tile([128, TS], FP16, name=f"e{dj}"))

            # vt16 (pass-2 fp16 values) trickle-loaded one tile per dj
            # iteration on gpsimd — needed only from the first pass-2 group.
            vt_tiles = []
            for kt in range(KT):
                vt = vt_pool.tile([128, TS], FP16, name=f"vt{kt}")
                vt_tiles.append(vt)

            qcol = const_pool.tile([128, KT], F32)   # q in [p, kt] layout
            # q in fp8, one value per 16B block: dual-fp8 LDWEIGHTS requires
            # the pair stride to be even and 16B-aligned, so the (ks=0, ks=1)
            # elements of a DoubleRow pair sit 16 bytes apart.
            qcol16 = const_pool.tile([128, KT * 16], FP8)
            nc.scalar.dma_start(qcol[:, :], qfull[:].rearrange("(kt p) -> p kt", p=128))
            nc.vector.tensor_copy(
                qcol16[:, :].rearrange("p (a b) -> p a b", b=16)[:, :, 0:1],
                qcol[:, :].rearrange("p (a b) -> p a b", b=1))
            QW = min(512, D)
            QDC = D // QW

            def emit_w1_loads(w1_pool):
                # W1 streams on scalar (sync carries vt8; gpsimd is poisoned
                # by the warmup collective) in HALF tiles
                # (0.25MB, 0.7us) with bufs=6, h-major so the matvec's pass A
                # (dc 0,1 = h0 halves) consumes loads in arrival order.
                tiles = {}
                for h in range(2):
                    for kt2 in range(KT2):
                        w1tile = w1_pool.tile([128, 2, D // 2], FP8, tag="w1t")
                        nc.scalar.dma_start(
                            w1tile[:, :, :],
                            w1t8[kt2, :, :, h * (D // 2):(h + 1) * (D // 2)])
                        tiles[(kt2, h)] = w1tile
                return tiles

            def emit_matvec(w1_tiles, qrow_pool):
                # q_proj row = sum_kt2 q_pair.T @ W1T[pair] in fp8 DoubleRow
                # (softmax is nearly invariant to the per-column q_proj
                # quantization error), then transpose the row into the
                # per-partition [p, dj] layout.
                # Two dc-passes so the pq accumulators fit in the dedicated
                # 2-bank psum2 pool — sharing the mains' psum pool here
                # couples the matvec to dj0's activations and wedges the
                # whole early phase.
                qrow = qrow_pool.tile([1, D], F32, name="qrow")  # q_proj row
                NDCQ = D // 2 // QW
                for h in range(2):
                    pq_tiles = [psum2_pool.tile([1, QW], F32, name=f"pq{h}{i}",
                                                tag="pT")
                                for i in range(NDCQ)]
                    for kt2 in range(KT2):
                        qpair = qcol16[:, :].rearrange(
                            "p (a b) -> p a b", b=16)[:, 2 * kt2:2 * kt2 + 2, 0:1]
                        half = w1_tiles[(kt2, h)]
                        for dcq in range(NDCQ):
                            nc.tensor.matmul(
                                pq_tiles[dcq][:, :],
                                qpair,
                                half[:, :, dcq * QW:(dcq + 1) * QW],
                                start=(kt2 == 0), stop=(kt2 == KT2 - 1),
                                perf_mode=DR)
                    for dcq in range(NDCQ):
                        dc = h * NDCQ + dcq
                        nc.scalar.activation(
                            qrow[:, dc * QW:(dc + 1) * QW], pq_tiles[dcq][:, :],
                            mybir.ActivationFunctionType.Copy,
                            bias=0.0, scale=INV_W_SCALE)
                pqt = psum2_pool.tile([128, DT], F32, name="pqt", tag="pT")
                for dj in range(DT):
                    nc.tensor.transpose(
                        pqt[:, dj:dj + 1],
                        qrow[:, dj * 128:(dj + 1) * 128], ones1[:, 0:1])
                nc.scalar.copy(qbv[:, :], pqt[:, :])

                # biases / v in [p, dj] layout: elem (p, j) <- dram[128j + p]
                nc.scalar.dma_start(b1v[:, :], w1b[:].rearrange("(j p) -> p j", p=128))
                nc.scalar.dma_start(b2v[:, :], w2b[:].rearrange("(j p) -> p j", p=128))
                nc.scalar.dma_start(vv[:, :], vvec[:].rearrange("(j p) -> p j", p=128))
                nc.vector.tensor_add(b1v[:, :], b1v[:, :], b2v[:, :])
                nc.vector.tensor_add(qbv[:, :], qbv[:, :], b1v[:, :])

            ndma_state = [0]

            def emit_group(djs, tail3=False):
                # pass-2 pipeline for a list of dj tiles (one contiguous
                # output chunk): outT = e * valuesT/64 on DVE (both fp16,
                # e unscaled), then a regular fp16 matmul against the scaled
                # diagonal M_dj = diag(64/S[d]) transposes AND applies the
                # softmax normalization in one PE op, writing f32 PSUM
                # (recycling the pass-1 psum pool — mains are done by now).
                # Evacuation is a pure f32 copy alternating Scalar/Vector
                # into a per-th staging tile; ONE batched DMA per th writes
                # 512 output rows (4 itl blocks) with a single descriptor.
                nj = len(djs)
                d0 = djs[0]
                nitl = THW // 128
                mds = []
                for dj in djs:
                    md = mdiag_pool.tile([128, 128], FP16, tag="md", name="md")
                    nc.vector.tensor_scalar(
                        out=md[:, :], in0=ident16[:, :],
                        scalar1=rv2[:, dj:dj + 1], scalar2=None,
                        op0=mybir.AluOpType.mult)
                    mds.append(md)
                for th in range(NTH):
                    oT = []
                    for jj in range(nj):
                        dj = djs[jj]
                        ot = outT_pool.tile([128, THW], FP16, tag="oT", name="ot")
                        nc.vector.tensor_mul(
                            ot[:, :],
                            e_tiles[dj][:, th * THW:(th + 1) * THW],
                            vt_tiles[dj][:, th * THW:(th + 1) * THW])
                        oT.append(ot)
                    osb = osb_pool.tile([128, nitl * nj * 128], F32,
                                        name="osb", tag="osb")
                    for itl in range(nitl):
                        pso = psum_pool.tile([128, 512], F32, tag="ps",
                                             name="pso")
                        for jj in range(nj):
                            nc.tensor.matmul(
                                pso[:, jj * 128:(jj + 1) * 128],
                                oT[jj][:, itl * 128:(itl + 1) * 128],
                                mds[jj][:, :],
                                start=True, stop=True)
                        ndma_state[0] += 1
                        oslice = osb[:, itl * nj * 128:(itl + 1) * nj * 128]
                        if ndma_state[0] % 2:
                            nc.scalar.copy(oslice, pso[:, :nj * 128])
                        else:
                            nc.vector.tensor_copy(oslice, pso[:, :nj * 128])
                    nc.sync.dma_start(
                        out[th * THW:(th + 1) * THW,
                            d0 * 128:(d0 + nj) * 128].rearrange(
                                "(a p) f -> p a f", p=128),
                        osb[:, :].rearrange("p (a f) -> p a f", a=nitl))

            # sum-exp AllReduce split points, sized so each AR's ~35us ncfw
            # latency hides under remaining pass-1 matmuls or earlier groups'
            # pass-2 work.  Pass-2 groups are lists of dj indices per output
            # chunk.
            if DT >= 16:
                # Two AR parts: A (dj 0-11) triggers mid-pass-1 and lands
                # before the tail starts; B (dj 12-15) triggers at the last
                # exp and lands while G0-G2 (~35us of evac-paced pass-2)
                # still run — near-zero exposed latency.  No mid-mains
                # interleaving: the chip is power-throttled, so overlapping
                # pass-2 with pass-1 just lowers the PE clock.
                ar_parts = [(0, 8), (8, 12), (12, 16)]
                groups = [list(range(4 * g, 4 * g + 4)) for g in range(3)] + \
                         [[12, 13], [14, 15]]
                interleave_at = {}
                readback_at = {8: 0, 12: 1}
                mid_groups = [groups[0], groups[1], groups[2]]
                b_groups = [groups[3], groups[4]]
            else:
                ar_parts = [(0, DH)] + ([(DH, DT)] if DH < DT else [])
                groups = [list(range(g * GJ, (g + 1) * GJ)) for g in range(NG)]
                interleave_at = {}
                readback_at = {}
                mid_groups = [g for g in groups if g[-1] < DH]
                b_groups = [g for g in groups if g[-1] >= DH]

            s_bounce = []
            for pi, (lo, hi) in enumerate(ar_parts):
                sin = dram_pool.tile([128, hi - lo], F32, name=f"s_in{pi}")
                sout = dram_pool.tile([128, hi - lo], F32, name=f"s_out{pi}")
                s_bounce.append((sin, sout))

            def ar_trigger(pi):
                lo, hi = ar_parts[pi]
                sin, sout = s_bounce[pi]
                nc.gpsimd.dma_start(sin[:, :], Sloc[:, lo:hi])
                nc.gpsimd.collective_compute(
                    "AllReduce", mybir.AluOpType.add,
                    replica_groups=[list(range(N_CORES_))],
                    ins=[sin.opt()], outs=[sout.opt()],
                )

            def ar_readback_dma(pi):
                # Inline on gpsimd right after the matching collective: it
                # head-of-line blocks only LATER collective machinery, which
                # is gated on later data anyway.
                lo, hi = ar_parts[pi]
                sin, sout = s_bounce[pi]
                nc.gpsimd.dma_start(rv2[:, lo:hi], sout[:, :])

            def ar_readback_arith(pi):
                # rv2 = 64 / S  (64/S ~ 0.004 keeps the pass-2 diagonal well
                # inside fp16 normals; vt16 is pre-scaled by 1/64 on host).
                # Emitted post-loop so it never head-of-line blocks the
                # vector queue's pass-1 reduces.
                lo, hi = ar_parts[pi]
                nc.vector.tensor_scalar_mul(rv2[:, lo:hi], rv2[:, lo:hi],
                                            0.015625)
                nc.vector.reciprocal(rv2[:, lo:hi], rv2[:, lo:hi])

            def ar_readback(pi):
                ar_readback_dma(pi)
                ar_readback_arith(pi)

            def emit_act(dj, srcs):
                # tanh per 512-wide psum bank, then ONE 2048-wide exp whose
                # accum_out IS the local softmax denominator (written straight
                # into Sloc — no separate reduce).
                st = st_pool.tile([128, TS], FP16, name="st", tag="st")
                for tc_i in range(TC):
                    nc.scalar.activation(
                        st[:, tc_i * 512:(tc_i + 1) * 512], srcs[tc_i][:, :],
                        mybir.ActivationFunctionType.Tanh,
                        bias=qbv[:, dj:dj + 1], scale=INV_W_SCALE,
                    )
                nc.scalar.activation(
                    e_tiles[dj][:, :], st[:, :],
                    mybir.ActivationFunctionType.Exp,
                    bias=0.0, scale=vv[:, dj:dj + 1],
                    accum_out=Sloc[:, dj:dj + 1],
                )
                inloop_parts = ar_parts[:-1] if len(ar_parts) > 1 else ar_parts
                for pi, (lo, hi) in enumerate(inloop_parts):
                    if dj == hi - 1:
                        ar_trigger(pi)
                        if not readback_at:
                            ar_readback(pi)
                if dj in readback_at:
                    ar_readback_dma(readback_at[dj])

            # The q-projection matvec runs FIRST: its W1 tiles stream on
            # gpsimd while sync streams vt8, so the matvec's ~11us on the
            # tensor queue overlaps the vt8 load that dj0 would have waited
            # for anyway.  qbv is ready before dj0's activations, so no
            # PSUM staging is needed.
            w1_ctx = tc.tile_pool(name="w1pool", bufs=6)
            w1_pool = w1_ctx.__enter__()
            qrow_ctx = tc.tile_pool(name="qrowp", bufs=1)
            qrow_pool = qrow_ctx.__enter__()
            w1_tiles = emit_w1_loads(w1_pool)
            emit_matvec(w1_tiles, qrow_pool)
            qrow_ctx.__exit__(None, None, None)
            w1_ctx.__exit__(None, None, None)
            for dj in range(DT):
                if dj < len(w2tb_pre):
                    w2tb = w2tb_pre[dj]
                else:
                    w2tb = w2tb_pool.tile([128, KT, 128], FP8, tag="w2tb",
                                          name="w2tb")
                    nc.sync.dma_start(w2tb[:, :, :], w2t8[dj, :, :, :])
                # vt16 trickle (pass-2 fp16 values): two tiles per dj from
                # dj=2 on, so it never front-runs the vt8/W1 loads.
                if 2 <= dj < 2 + KT // 2:
                    for h in range(2):
                        kt = 2 * (dj - 2) + h
                        nc.sync.dma_start(
                            vt_tiles[kt][:, :],
                            valsT[kt * 128:(kt + 1) * 128, :])
                ps_tiles = [psum_pool.tile([128, 512], F32, tag="ps", name=f"ps{i}")
                            for i in range(TC)]
                # kt2 OUTER: stationary pair reused TC times; dj==0 streams
                # at vt8-DMA pace.  DoubleRow: 256-deep contraction per pass.
                # The last two djs run tc-OUTER instead, so their psum banks
                # complete (and free) incrementally — the tail's first
                # transpose blocks then don't wait on dj15's last matmul.
                if dj >= DT - 2:
                    for tc_i in range(TC):
                        for kt2 in range(KT2):
                            nc.tensor.matmul(
                                ps_tiles[tc_i][:, :],
                                w2tb[:, 2 * kt2:2 * kt2 + 2, :],
                                vt8_tiles[kt2][:, :, tc_i * 512:(tc_i + 1) * 512],
                                start=(kt2 == 0),
                                stop=(kt2 == KT2 - 1),
                                perf_mode=DR,
                            )
                else:
                    for kt2 in range(KT2):
                        for tc_i in range(TC):
                            nc.tensor.matmul(
                                ps_tiles[tc_i][:, :],
                                w2tb[:, 2 * kt2:2 * kt2 + 2, :],
                                vt8_tiles[kt2][:, :, tc_i * 512:(tc_i + 1) * 512],
                                start=(kt2 == 0),
                                stop=(kt2 == KT2 - 1),
                                perf_mode=DR,
                            )
                emit_act(dj, ps_tiles)
                for g in interleave_at.get(dj, []):
                    emit_group(g)

            vt8_ctx.__exit__(None, None, None)
            osb_ctx = tc.tile_pool(name="osb", bufs=2)
            osb_pool = osb_ctx.__enter__()
            outT_ctx = tc.tile_pool(name="outT", bufs=4)
            outT_pool = outT_ctx.__enter__()
            mdiag_ctx = tc.tile_pool(name="mdiag", bufs=6)
            mdiag_pool = mdiag_ctx.__enter__()

            # ---------------- pass-2 tail ---------------
            # Trigger the final AR part right after dj15's local reduce,
            # then run the groups as their parts land: A1 (triggered at dj7)
            # is long done, A2 (dj11) lands while G0/G1 run, A3 lands while
            # G2 runs.
            if len(ar_parts) > 1:
                ar_trigger(len(ar_parts) - 1)

            if readback_at:
                ar_readback_arith(0)
                emit_group(mid_groups[0])
                emit_group(mid_groups[1])
                ar_readback_arith(1)
                for g in mid_groups[2:]:
                    emit_group(g)
                ar_readback(len(ar_parts) - 1)
                for g in b_groups:
                    emit_group(g, tail3=True)
            else:
                if len(ar_parts) > 1:
                    ar_readback(len(ar_parts) - 1)
                for g in mid_groups + b_groups:
                    emit_group(g)

            mdiag_ctx.__exit__(None, None, None)
            outT_ctx.__exit__(None, None, None)
            osb_ctx.__exit__(None, None, None)

    nc.compile()
    return nc


_NC_CACHE = None


def _get_nc():
    global _NC_CACHE
    if _NC_CACHE is None:
        _NC_CACHE = build_kernel()
    return _NC_CACHE


def make_in_maps(query, values, v, W1_w, W1_b, W2_w, W2_b,
                 D_=None, TS_=None, n_cores=N_CORES):
    import ml_dtypes
    D_ = D_ or D
    TS_ = TS_ or TS
    DT_ = D_ // 128
    KT_ = D_ // 128
    KT2_ = KT_ // 2
    fp8 = ml_dtypes.float8_e4m3
    # W1T DoubleRow pairs: [kt2, p, ks, d] = 64*W1_w[d, 256kt2+128ks+p]
    w1t_blocked = np.ascontiguousarray(
        (W1_w.T * W_SCALE).reshape(KT2_, 2, 128, D_).transpose(0, 2, 1, 3)
        .astype(fp8))
    # w2t blocked: B[dj, p, kt, f] = 64*W2_w[128dj+f, 128kt+p]
    # (pairs of k-blocks are adjacent along the kt dim => DoubleRow-ready)
    w2t_blocked = np.ascontiguousarray(
        (W2_w * W_SCALE).reshape(DT_, 128, KT_, 128).transpose(0, 3, 2, 1)
        .astype(fp8))
    in_maps = []
    for c in range(n_cores):
        vs = np.ascontiguousarray(values[c * TS_:(c + 1) * TS_])
        # vt16 carries values/64 (exact power-of-2 scale); the pass-2
        # diagonal is 64/S so the product is values * w.
        vsT = np.ascontiguousarray((vs.T * (1.0 / 64.0)).astype(np.float16))
        vsT8 = np.ascontiguousarray(
            vs.T.astype(fp8).reshape(KT2_, 2, 128, TS_).transpose(0, 2, 1, 3))
        in_maps.append({
            "valsT": vsT,
            "valsT8": vsT8,
            "w2t8": w2t_blocked,
            "w1t8": w1t_blocked,
            "qfull": query,
            "w1b": W1_b,
            "w2b": W2_b,
            "vvec": v,
        })
    return in_maps


def kernel(query, values, v, W1_w, W1_b, W2_w, W2_b, _trace=False, _trace_kwargs=None):
    query = np.asarray(query, np.float32)
    values = np.asarray(values, np.float32)
    v = np.asarray(v, np.float32)
    W1_w = np.asarray(W1_w, np.float32)
    W1_b = np.asarray(W1_b, np.float32)
    W2_w = np.asarray(W2_w, np.float32)
    W2_b = np.asarray(W2_b, np.float32)

    nc = _get_nc()
    in_maps = make_in_maps(query, values, v, W1_w, W1_b, W2_w, W2_b)
    res = run_bass_kernel_spmd(
        nc, in_maps, core_ids=list(range(N_CORES)),
        trace=_trace, **(_trace_kwargs or {}),
    )
    shards = [np.asarray(om["out"], np.float32) for om in res.results]
    out = np.concatenate(shards, axis=0)
    if _trace:
        return out, res
    return out
